# revision 22
# baseline (speedup 1.0000x reference)
"""DeepseekV32 MLA-style attention on 8 Trainium2 NeuronCores (Bass/Tile).

Sharding: row shard (256 rows/core) for the low-rank A projections and q_b;
head shard (2 heads/core) for kv_b expansion + attention + o_proj.  Host
prep: hidden is pre-transposed (hsT input), rope columns of wkv_a / wq_b are
pre-permuted so the de-interleave is free, wo is sliced per-core by head,
layernorm weights and softmax scale are folded into the B projections.

Exactly two collectives (they serialize on the collective engine): AllGather
of X^T=[ckv_normed; roped k_rot] and AllToAll of q (row-shard ->
head-shard).  The output projection is computed per-head (a partial over all
rows) and the 8 partials are summed on the host, which removes the output
collective entirely.

Attention computes scores TRANSPOSED (k on partitions, q on free dim):
probsT = exp(scoresT) feeds the PV matmul directly (no transposes, no
PSUM->SBUF probs copies).  The softmax denominator comes from a ones-vector
matmul accumulated alongside PV; normalization is applied while draining
attT via partition_broadcast of the reciprocal.  o_proj is interleaved into
the attention stream as each 512-column block of both heads completes.

All matmuls run in bf16 with fp32 PSUM accumulation; softmax and rmsnorm
statistics are fp32.
"""
import sys

sys.path.insert(0, "/opt/trn_rl_repo")

import numpy as np
import ml_dtypes
from contextlib import ExitStack

import concourse.bass as bass
import concourse.tile as tile
import concourse.mybir as mybir
from concourse import bacc
from concourse.masks import make_identity
from bass_rust import add_dep_helper
from concourse.bass_utils import run_bass_kernel_spmd

BF16 = mybir.dt.bfloat16
F32 = mybir.dt.float32
AF = mybir.ActivationFunctionType

NC = 8            # cores
B, S, H = 1, 2048, 2048
NH = 16           # heads
QLR = 1536        # q lora rank
KVLR = 512        # kv lora rank
DR = 64           # rope dim
DN = 128          # nope dim
DV = 128          # v dim
DQK = DN + DR     # 192
EPS = 1e-6
HPC = NH // NC    # heads per core = 2
SPC = S // NC     # seq rows per core = 256
ST = SPC // 128   # row tiles per core = 2
NEG = -1e30       # causal mask fill
NKT = KVLR // 128  # 4
HW = DR // 2      # 32

_CACHED = {}


def _ts(i, n):
    return slice(i * n, (i + 1) * n)


def _chunked(dram_ap, nchunk, rows, cols):
    """AP reading `nchunk` consecutive [rows, cols] row-blocks of a 2-D dram
    tensor as one [rows, nchunk, cols] transfer."""
    return bass.AP(tensor=dram_ap.tensor, offset=0,
                   ap=[[cols, rows], [rows * cols, nchunk], [1, cols]])


def build():
    nc = bacc.Bacc("TRN2", target_bir_lowering=False, debug=False,
                   num_devices=NC)

    # ---- kernel I/O (per-core shards / replicated weights) ----
    hsT_d = nc.dram_tensor("hsT", [H, SPC], BF16, kind="ExternalInput").ap()
    cos_d = nc.dram_tensor("cosr", [SPC, DR], F32, kind="ExternalInput").ap()
    sin_d = nc.dram_tensor("sinr", [SPC, DR], F32, kind="ExternalInput").ap()
    wqa_d = nc.dram_tensor("wqa", [H, QLR], BF16, kind="ExternalInput").ap()
    wkva_d = nc.dram_tensor("wkva", [H, KVLR + DR], BF16,
                            kind="ExternalInput").ap()
    wqb_d = nc.dram_tensor("wqb", [QLR, NH * DQK], BF16,
                           kind="ExternalInput").ap()
    wkvb_d = nc.dram_tensor("wkvb", [KVLR, HPC * (DN + DV)], BF16,
                            kind="ExternalInput").ap()
    wo_d = nc.dram_tensor("wo", [HPC * DV, H], BF16, kind="ExternalInput").ap()
    # per-core output: partial o_proj (this core's 2 heads) over ALL rows
    out_d = nc.dram_tensor("out", [S, H], BF16, kind="ExternalOutput").ap()

    # ---- collective buffers ----
    ag_in = nc.dram_tensor("ag_in", [KVLR + DR, SPC], BF16).ap()
    ag_out = nc.dram_tensor("ag_out", [NC * (KVLR + DR), SPC], BF16,
                            addr_space="Shared").ap()
    a2aq_in = nc.dram_tensor("a2aq_in", [S, HPC * DQK], BF16).ap()
    a2aq_out = nc.dram_tensor("a2aq_out", [S, HPC * DQK], BF16).ap()

    rg = [list(range(NC))]
    nh = H // 128

    with tile.TileContext(nc) as tc, ExitStack() as ctx:
        singles = ctx.enter_context(tc.tile_pool(name="singles", bufs=1))
        small = ctx.enter_context(tc.tile_pool(name="small", bufs=4))

        ident = singles.tile([128, 128], BF16)
        make_identity(nc, ident)
        eps_t = singles.tile([128, 1], F32)
        nc.vector.memset(eps_t, float(EPS))
        ones_bf = singles.tile([128, 1], BF16)
        nc.vector.memset(ones_bf, 1.0)
        # cmaskT[k, q] = 0 if k <= q else NEG (diagonal block of scoresT)
        cmaskT = singles.tile([128, 128], F32)
        nc.gpsimd.memset(cmaskT, 0.0)
        nc.gpsimd.affine_select(
            out=cmaskT, in_=cmaskT, compare_op=mybir.AluOpType.is_ge,
            fill=NEG, base=0, pattern=[[1, 128]], channel_multiplier=-1)

        hsT_sb = []
        wkva_sb = []
        wop = ctx.enter_context(tc.tile_pool(name="wop", bufs=1))
        interph = ctx.enter_context(tc.tile_pool(name="interph", bufs=1))

        # =========== phase 1: ckv -> X^T -> AllGather ===========
        with tc.tile_pool(name="ph1", bufs=1) as ph1, \
             tc.tile_pool(name="ps1", bufs=2, space="PSUM") as ps1:
            # batched input DMAs (one transfer per half-tensor; the SP
            # queue's per-DMA dispatch cost would otherwise pace the phase)
            wkva_all = ph1.tile([128, nh, KVLR + DR], BF16, tag="wkva",
                                name="wkva_all")
            hsT_all = interph.tile([128, nh, SPC], BF16, tag="hsT",
                                   name="hsT_all")
            for ch in range(2):
                hh = nh // 2
                src = bass.AP(tensor=hsT_d.tensor, offset=ch * hh * 128 * SPC,
                              ap=[[SPC, 128], [128 * SPC, hh], [1, SPC]])
                nc.sync.dma_start(out=hsT_all[:, _ts(ch, hh), :], in_=src)
                src = bass.AP(tensor=wkva_d.tensor,
                              offset=ch * hh * 128 * (KVLR + DR),
                              ap=[[KVLR + DR, 128], [128 * (KVLR + DR), hh],
                                  [1, KVLR + DR]])
                nc.sync.dma_start(out=wkva_all[:, _ts(ch, hh), :], in_=src)
            wkva_sb = [wkva_all[:, ht, :] for ht in range(nh)]
            hsT_sb = [hsT_all[:, ht, :] for ht in range(nh)]

            cos_all = singles.tile([128, ST, DR], F32, name="cos_all")
            sin_all = singles.tile([128, ST, DR], F32, name="sin_all")
            nc.sync.dma_start(out=cos_all, in_=_chunked(cos_d, ST, 128, DR))
            nc.sync.dma_start(out=sin_all, in_=_chunked(sin_d, ST, 128, DR))
            cos_sb = [cos_all[:, st, :] for st in range(ST)]
            sin_sb = [sin_all[:, st, :] for st in range(ST)]

            stage_dmas = []
            ckv_ps = [ps1.tile([128, KVLR], F32, tag=f"ckv{st}", bufs=1,
                               name="ckv_p") for st in range(ST)]
            rope_ps = [ps1.tile([128, DR], F32, tag=f"ckr{st}", bufs=1,
                                name="rope_p") for st in range(ST)]
            for ht in range(nh):
                for st in range(ST):
                    nc.tensor.matmul(out=ckv_ps[st],
                                     lhsT=hsT_sb[ht][:, _ts(st, 128)],
                                     rhs=wkva_sb[ht][:, 0:KVLR],
                                     start=(ht == 0), stop=(ht == nh - 1))
            for ht in range(nh):
                for st in range(ST):
                    nc.tensor.matmul(out=rope_ps[st],
                                     lhsT=hsT_sb[ht][:, _ts(st, 128)],
                                     rhs=wkva_sb[ht][:, KVLR:KVLR + DR],
                                     start=(ht == 0), stop=(ht == nh - 1))

            for st in range(ST):
                # rmsnorm over KVLR, stats straight off PSUM
                sq = small.tile([128, KVLR], F32, tag="sqscr", bufs=2,
                                name="sq")
                ssq = small.tile([128, 1], F32, tag="ssq", name="ssq")
                nc.scalar.activation(out=sq, in_=ckv_ps[st], func=AF.Square,
                                     accum_out=ssq)
                rstd = small.tile([128, 1], F32, tag="rstd", name="rstd")
                nc.scalar.activation(out=rstd, in_=ssq, func=AF.Sqrt,
                                     scale=1.0 / KVLR, bias=eps_t)
                nc.vector.reciprocal(out=rstd, in_=rstd)
                cn_t = ph1.tile([128, KVLR], BF16, tag=f"ckvn{st}",
                                name="cn_t")
                nc.vector.tensor_scalar_mul(cn_t, ckv_ps[st], rstd)
                # rope on k_rot (weights pre-permuted -> contiguous halves)
                kr_t = ph1.tile([128, DR], BF16, tag=f"krot{st}", name="kr_t")
                t0 = small.tile([128, HW], F32, tag="krs0", bufs=2, name="t0")
                t1 = small.tile([128, HW], F32, tag="krs1", bufs=2, name="t1")
                xe, xo = rope_ps[st][:, 0:HW], rope_ps[st][:, HW:DR]
                nc.vector.tensor_mul(t0, xe, cos_sb[st][:, 0:HW])
                nc.vector.tensor_mul(t1, xo, sin_sb[st][:, 0:HW])
                nc.vector.tensor_sub(kr_t[:, 0:HW], t0, t1)
                nc.vector.tensor_mul(t0, xo, cos_sb[st][:, HW:DR])
                nc.vector.tensor_mul(t1, xe, sin_sb[st][:, HW:DR])
                nc.vector.tensor_add(kr_t[:, HW:DR], t0, t1)

                # transpose [128, 512+64] -> X^T, stage with 2 DMAs per st
                xp = ps1.tile([128, KVLR], BF16, tag="xp", name="xp")
                for kt in range(NKT):
                    nc.tensor.transpose(out=xp[:, _ts(kt, 128)],
                                        in_=cn_t[:, _ts(kt, 128)],
                                        identity=ident)
                x_t = small.tile([128, NKT, 128], BF16, tag="xT", bufs=2,
                                 name="x_t")
                if st == 0:
                    nc.vector.tensor_copy(out=x_t,
                                          in_=xp.rearrange(
                                              "p (k c) -> p k c", k=NKT))
                else:
                    nc.scalar.copy(out=x_t,
                                   in_=xp.rearrange("p (k c) -> p k c",
                                                    k=NKT))
                dst = bass.AP(tensor=ag_in.tensor, offset=st * 128,
                              ap=[[SPC, 128], [128 * SPC, NKT], [1, 128]])
                stage_dmas.append(nc.gpsimd.dma_start(out=dst, in_=x_t))
                pr_t = ps1.tile([64, 128], BF16, tag="tpr", name="pr_t")
                nc.tensor.transpose(out=pr_t, in_=kr_t, identity=ident)
                xr_t = small.tile([64, 128], BF16, tag="xTr", name="xr_t")
                nc.vector.tensor_copy(out=xr_t, in_=pr_t)
                stage_dmas.append(
                    nc.gpsimd.dma_start(
                        out=ag_in[KVLR:KVLR + DR, _ts(st, 128)], in_=xr_t))

            # ---- collective 1: AllGather X^T ----
            nc.gpsimd.collective_compute(
                "AllGather", mybir.AluOpType.bypass, replica_groups=rg,
                ins=[ag_in.opt()], outs=[ag_out.opt()])

        # =========== phase 2: cq -> q_b -> rope -> AllToAll q ===========
        wqb_stack = ExitStack()
        wqbp = wqb_stack.enter_context(tc.tile_pool(name="wqbp", bufs=1))
        wkvb_sb = []
        wo_sb = []
        with tc.tile_pool(name="ph2", bufs=1) as ph2:
            # wqa in 4 chunks so cq overlaps the transfer
            wqa_sb = []
            for cg in range(4):
                wq_t = ph2.tile([128, 4, QLR], BF16, tag=f"wqa{cg}",
                                name="wq_t")
                src = bass.AP(tensor=wqa_d.tensor, offset=cg * 4 * 128 * QLR,
                              ap=[[QLR, 128], [128 * QLR, 4], [1, QLR]])
                d = nc.sync.dma_start(out=wq_t, in_=src)
                for s in stage_dmas:
                    add_dep_helper(d.ins, s.ins, reason="bus order")
                wqa_sb += [wq_t[:, i, :] for i in range(4)]
            wkvb_all = interph.tile([128, NKT, HPC * (DN + DV)], BF16,
                                    tag="wkvb", name="wkvb_all")
            d = nc.sync.dma_start(out=wkvb_all,
                                  in_=_chunked(wkvb_d, NKT, 128,
                                               HPC * (DN + DV)))
            for s in stage_dmas:
                add_dep_helper(d.ins, s.ins, reason="bus order")
            wkvb_sb = [wkvb_all[:, kt, :] for kt in range(NKT)]
            wqb_sb = []
            for cg in range(6):
                wb_t = wqbp.tile([128, 2, NH * DQK], BF16, tag=f"wqb{cg}",
                                 name="wb_t")
                src = bass.AP(tensor=wqb_d.tensor,
                              offset=cg * 2 * 128 * NH * DQK,
                              ap=[[NH * DQK, 128], [128 * NH * DQK, 2],
                                  [1, NH * DQK]])
                d = nc.sync.dma_start(out=wb_t, in_=src)
                for s in stage_dmas:
                    add_dep_helper(d.ins, s.ins, reason="bus order")
                wqb_sb += [wb_t[:, i, :] for i in range(2)]
            wo_all = wop.tile([128, HPC, H], BF16, tag="wo", name="wo_all")
            d = nc.sync.dma_start(out=wo_all,
                                  in_=_chunked(wo_d, HPC, 128, H))
            for s in stage_dmas:
                add_dep_helper(d.ins, s.ins, reason="bus order")
            wo_sb = [wo_all[:, h, :] for h in range(HPC)]

            # cq: ht outer (stream wqa), both row-tiles in parallel
            s2a = ExitStack()
            ps2a = s2a.enter_context(tc.tile_pool(name="ps2a", bufs=1,
                                                  space="PSUM"))
            cq_ps = [[ps2a.tile([128, 512], F32, tag=f"cq{st}_{rb}", bufs=1,
                                name="cq_p") for rb in range(QLR // 512)]
                     for st in range(ST)]
            for ht in range(nh):
                for st in range(ST):
                    for rb in range(QLR // 512):
                        nc.tensor.matmul(out=cq_ps[st][rb],
                                         lhsT=hsT_sb[ht][:, _ts(st, 128)],
                                         rhs=wqa_sb[ht][:, _ts(rb, 512)],
                                         start=(ht == 0), stop=(ht == nh - 1))
            cqn_bf = []
            for st in range(ST):
                ssqs = []
                for rb in range(QLR // 512):
                    sqq = small.tile([128, 512], F32, tag="sqq", bufs=2,
                                     name="sqq")
                    ssq = small.tile([128, 1], F32, tag="ssq3", bufs=6,
                                     name="ssq")
                    nc.scalar.activation(out=sqq, in_=cq_ps[st][rb],
                                         func=AF.Square, accum_out=ssq)
                    ssqs.append(ssq)
                nc.vector.tensor_add(ssqs[0], ssqs[0], ssqs[1])
                nc.vector.tensor_add(ssqs[0], ssqs[0], ssqs[2])
                rstd = small.tile([128, 1], F32, tag="rstd", name="rstd")
                nc.scalar.activation(out=rstd, in_=ssqs[0], func=AF.Sqrt,
                                     scale=1.0 / QLR, bias=eps_t)
                nc.vector.reciprocal(out=rstd, in_=rstd)
                cn_t = ph2.tile([128, QLR], BF16, tag=f"cqn{st}", name="cn_t")
                for rb in range(QLR // 512):
                    nc.vector.tensor_scalar_mul(cn_t[:, _ts(rb, 512)],
                                                cq_ps[st][rb], rstd)
                cqn_bf.append(cn_t)

            # transpose cqn -> cqnT [1536, 256] (batched drains, alternating
            # engines)
            cqnT = []
            for rt in range(QLR // 128):
                cT_t = ph2.tile([128, SPC], BF16, tag=f"cqnT{rt}",
                                name="cT_t")
                p_t = ps2a.tile([128, SPC], BF16, tag="tp", bufs=2,
                                name="p_t")
                for st in range(ST):
                    nc.tensor.transpose(out=p_t[:, _ts(st, 128)],
                                        in_=cqn_bf[st][:, _ts(rt, 128)],
                                        identity=ident)
                if rt % 2 == 0:
                    nc.scalar.copy(out=cT_t, in_=p_t)
                else:
                    nc.vector.tensor_copy(out=cT_t, in_=p_t)
                cqnT.append(cT_t)
            s2a.close()

            # q_b per row-tile; psum in head-pair blocks of 384 cols so the
            # rope slicing never crosses a PSUM tile boundary
            s2b = ExitStack()
            ps2b = s2b.enter_context(tc.tile_pool(name="ps2b", bufs=1,
                                                  space="PSUM"))
            nr = QLR // 128
            for st in range(ST):
                q_ps = [ps2b.tile([128, HPC * DQK], F32, tag=f"qb{nb}",
                                  bufs=1, name="q_p") for nb in range(NC)]
                for rt in range(nr):
                    for nb in range(NC):
                        nc.tensor.matmul(out=q_ps[nb],
                                         lhsT=cqnT[rt][:, _ts(st, 128)],
                                         rhs=wqb_sb[rt][:, _ts(nb, HPC * DQK)],
                                         start=(rt == 0), stop=(rt == nr - 1))
                # rope + bf16 pack: nope copies on Act, rope muls on DVE
                q_bf = ph2.tile([128, NH, DQK], BF16, tag=f"qbf{st}",
                                name="q_bf")
                for nb in range(NC):
                    qv = q_ps[nb].rearrange("p (h d) -> p h d", h=HPC)
                    dst = q_bf[:, nb * HPC:(nb + 1) * HPC, :]
                    nc.scalar.copy(out=dst[:, :, 0:DN], in_=qv[:, :, 0:DN])

                    def _bc(t, lo, hi):
                        return bass.AP(
                            tensor=t.tensor, offset=t.offset + lo,
                            ap=[list(t.ap[0]), [0, HPC], [1, hi - lo]])
                    cs, sn = cos_sb[st], sin_sb[st]
                    xe, xo = qv[:, :, DN:DN + HW], qv[:, :, DN + HW:DQK]
                    t0 = small.tile([128, HPC, HW], F32, tag="qrs0", bufs=2,
                                    name="t0")
                    t1 = small.tile([128, HPC, HW], F32, tag="qrs1", bufs=2,
                                    name="t1")
                    nc.vector.tensor_mul(t0, xe, _bc(cs, 0, HW))
                    nc.vector.tensor_mul(t1, xo, _bc(sn, 0, HW))
                    nc.vector.tensor_sub(dst[:, :, DN:DN + HW], t0, t1)
                    nc.vector.tensor_mul(t0, xo, _bc(cs, HW, DR))
                    nc.vector.tensor_mul(t1, xe, _bc(sn, HW, DR))
                    nc.vector.tensor_add(dst[:, :, DN + HW:DQK], t0, t1)
                # stage the whole row-tile with one DMA (8 dest chunks)
                dst = bass.AP(tensor=a2aq_in.tensor,
                              offset=st * 128 * HPC * DQK,
                              ap=[[HPC * DQK, 128], [SPC * HPC * DQK, NC],
                                  [1, HPC * DQK]])
                nc.gpsimd.dma_start(out=dst, in_=q_bf)
            s2b.close()
            # ---- collective 2: AllToAll q ----
            nc.gpsimd.collective_compute(
                "AllToAll", mybir.AluOpType.bypass, replica_groups=rg,
                ins=[a2aq_in.opt()], outs=[a2aq_out.opt()])
        wqb_stack.close()

        # =========== phase 3: k/v expansion + qT ===========
        with tc.tile_pool(name="ph3", bufs=1) as ph3, \
             tc.tile_pool(name="ph3b", bufs=4) as ph3b:
            s3 = ExitStack()
            ps3 = s3.enter_context(tc.tile_pool(name="ps3", bufs=2,
                                                space="PSUM"))
            krT = ph3.tile([64, NC, SPC], BF16, tag="krT", name="krT")
            src = bass.AP(tensor=ag_out.tensor, offset=KVLR * SPC,
                          ap=[[SPC, 64], [(KVLR + DR) * SPC, NC], [1, SPC]])
            nc.sync.dma_start(out=krT, in_=src)
            krTf = krT.rearrange("p g c -> p (g c)")

            kT = [ph3.tile([128, S], BF16, tag=f"kT{h}", name="kT_t")
                  for h in range(HPC)]
            v_sb = [[ph3.tile([128, DV], BF16, tag=f"v{h}_{kc}", name="v_t")
                     for kc in range(S // 128)] for h in range(HPC)]
            for g in range(NC):
                xk_t = ph3b.tile([128, NKT, SPC], BF16, tag="xk", bufs=3,
                                 name="xk_t")
                src = bass.AP(tensor=ag_out.tensor,
                              offset=g * (KVLR + DR) * SPC,
                              ap=[[SPC, 128], [128 * SPC, NKT], [1, SPC]])
                nc.sync.dma_start(out=xk_t, in_=src)
                xk = [xk_t[:, kt, :] for kt in range(NKT)]
                kps = [ps3.tile([128, SPC], F32, tag="mmk", bufs=2, name="kp")
                       for _ in range(HPC)]
                for kt in range(NKT):
                    for h in range(HPC):
                        nc.tensor.matmul(
                            out=kps[h],
                            lhsT=wkvb_sb[kt][:, h * (DN + DV):
                                             h * (DN + DV) + DN],
                            rhs=xk[kt], start=(kt == 0), stop=(kt == NKT - 1))
                for h in range(HPC):
                    if h == 0:
                        nc.scalar.copy(out=kT[h][:, _ts(g, SPC)], in_=kps[h])
                    else:
                        nc.vector.tensor_copy(out=kT[h][:, _ts(g, SPC)],
                                              in_=kps[h])
                for sub in range(ST):
                    vps = [ps3.tile([128, DV], F32, tag="mmv", bufs=2,
                                    name="vp") for _ in range(HPC)]
                    for kt in range(NKT):
                        for h in range(HPC):
                            nc.tensor.matmul(
                                out=vps[h], lhsT=xk[kt][:, _ts(sub, 128)],
                                rhs=wkvb_sb[kt][:, h * (DN + DV) + DN:
                                                (h + 1) * (DN + DV)],
                                start=(kt == 0), stop=(kt == NKT - 1))
                    for h in range(HPC):
                        nc.vector.tensor_copy(out=v_sb[h][g * ST + sub],
                                              in_=vps[h])

            # q^T per head from the AllToAll (batched loads + drains)
            qTn = [ph3.tile([128, S], BF16, tag=f"qTn{h}", name="qTn_t")
                   for h in range(HPC)]
            qTr = [ph3.tile([64, S], BF16, tag=f"qTr{h}", name="qTr_t")
                   for h in range(HPC)]
            for q4 in range(4):
                qblk = ph3b.tile([128, 4, HPC * DQK], BF16, tag="qblk",
                                 bufs=2, name="qblk")
                src = bass.AP(tensor=a2aq_out.tensor,
                              offset=q4 * 4 * 128 * HPC * DQK,
                              ap=[[HPC * DQK, 128], [128 * HPC * DQK, 4],
                                  [1, HPC * DQK]])
                nc.sync.dma_start(out=qblk, in_=src)
                for h in range(HPC):
                    pn = ps3.tile([128, 512], BF16, tag="tqn", bufs=2,
                                  name="pn")
                    pr = ps3.tile([64, 512], BF16, tag="tqr", bufs=2,
                                  name="pr")
                    for i in range(4):
                        nc.tensor.transpose(
                            out=pn[:, _ts(i, 128)],
                            in_=qblk[:, i, h * DQK:h * DQK + DN],
                            identity=ident)
                        nc.tensor.transpose(
                            out=pr[:, _ts(i, 128)],
                            in_=qblk[:, i, h * DQK + DN:(h + 1) * DQK],
                            identity=ident)
                    if h == 0:
                        nc.scalar.copy(out=qTn[h][:, _ts(q4, 512)], in_=pn)
                        nc.vector.tensor_copy(out=qTr[h][:, _ts(q4, 512)],
                                              in_=pr)
                    else:
                        nc.vector.tensor_copy(out=qTn[h][:, _ts(q4, 512)],
                                              in_=pn)
                        nc.scalar.copy(out=qTr[h][:, _ts(q4, 512)], in_=pr)
            s3.close()

            # ====== phase 4: attention (scoresT) + interleaved o_proj ======
            QB = 512
            NQB = S // QB
            attTn = [[None] * NQB for _ in range(HPC)]

            def oproj(qb, ps_pool, o_pool):
                """o_proj for q rows [qb*512, (qb+1)*512): both heads."""
                for sub in range(4):
                    qs = qb * 4 + sub
                    o_t = o_pool.tile([128, H], BF16, tag="osb", bufs=3,
                                      name="o_t")
                    for cb in range(H // 512):
                        op = ps_pool.tile([128, 512], F32, tag="op", bufs=2,
                                          name="op")
                        for h in range(HPC):
                            nc.tensor.matmul(
                                out=op,
                                lhsT=attTn[h][qb][:, _ts(sub, 128)],
                                rhs=wo_sb[h][:, _ts(cb, 512)],
                                start=(h == 0), stop=(h == HPC - 1))
                        if cb % 2 == 0:
                            nc.scalar.copy(out=o_t[:, _ts(cb, 512)], in_=op)
                        else:
                            nc.vector.tensor_copy(out=o_t[:, _ts(cb, 512)],
                                                  in_=op)
                    nc.sync.dma_start(out=out_d[_ts(qs, 128), :], in_=o_t)

            with tc.tile_pool(name="ps5", bufs=1, space="PSUM") as ps5, \
                 tc.tile_pool(name="ph5", bufs=1) as ph5:
                for qb in range(NQB):
                    for h in range(HPC):
                        # previous block's o_proj slots between the two head
                        # chains: its inputs are long-ready, so PE streams
                        # through it with no dependency stalls
                        if h == 1 and qb > 0:
                            oproj(qb - 1, ps5, ph5)
                        attp = ps5.tile([128, QB], F32, tag="attT", bufs=2,
                                        name="attp")
                        denp = ps5.tile([1, QB], F32, tag="den", bufs=2,
                                        name="denp")
                        nkc = 4 * qb + 4
                        # software-pipelined: PV/den of kc trail the score
                        # matmuls of kc+1 so PE never waits on exp
                        probs = [None] * nkc

                        def scores(kc):
                            off = max(0, (kc - 4 * qb) * 128)
                            scp = ps5.tile([128, QB], F32, tag="scT", bufs=2,
                                           name="scp")
                            nc.tensor.matmul(
                                out=scp[:, off:QB],
                                lhsT=kT[h][:, _ts(kc, 128)],
                                rhs=qTn[h][:, qb * QB + off:(qb + 1) * QB],
                                start=True, stop=False)
                            nc.tensor.matmul(
                                out=scp[:, off:QB],
                                lhsT=krTf[:, _ts(kc, 128)],
                                rhs=qTr[h][:, qb * QB + off:(qb + 1) * QB],
                                start=False, stop=True)
                            if kc >= 4 * qb:
                                nc.vector.tensor_add(scp[:, off:off + 128],
                                                     scp[:, off:off + 128],
                                                     cmaskT)
                            pt = ph3b.tile([128, QB], BF16, tag="probsT",
                                           bufs=4, name="probsT")
                            if off > 0:
                                nc.vector.memset(pt[:, 0:off], 0.0)
                            nc.scalar.activation(out=pt[:, off:QB],
                                                 in_=scp[:, off:QB],
                                                 func=AF.Exp)
                            probs[kc] = pt

                        def pv(kc):
                            nc.tensor.matmul(out=attp, lhsT=v_sb[h][kc],
                                             rhs=probs[kc],
                                             start=(kc == 0),
                                             stop=(kc == nkc - 1))
                            nc.tensor.matmul(out=denp, lhsT=ones_bf,
                                             rhs=probs[kc],
                                             start=(kc == 0),
                                             stop=(kc == nkc - 1))

                        scores(0)
                        for kc in range(1, nkc):
                            scores(kc)
                            pv(kc - 1)
                        pv(nkc - 1)

                        # normalize while draining attT
                        rec = small.tile([1, QB], F32, tag="rec", bufs=4,
                                         name="rec")
                        nc.vector.reciprocal(out=rec, in_=denp)
                        bca = small.tile([128, QB], F32, tag="bca", bufs=2,
                                         name="bca")
                        nc.gpsimd.partition_broadcast(bca, rec)
                        a_t = ph5.tile([128, QB], BF16, tag=f"attn{h}_{qb}",
                                       name="a_t")
                        nc.vector.tensor_mul(a_t, attp, bca)
                        attTn[h][qb] = a_t
                oproj(NQB - 1, ps5, ph5)

    nc.compile()
    return nc


def _prep(hidden_states, cos, sin, wq_a, q_ln, wq_b, wkv_a, kv_ln, wkv_b, wo):
    """Host-side sharding + weight prep: pre-transpose hidden, fold layernorm
    weights + softmax scale into the B projections, pre-permute rope columns
    (de-interleave), slice wo by head, cast to bf16."""
    bf = ml_dtypes.bfloat16
    hsT = np.ascontiguousarray(hidden_states.reshape(S, H).T.astype(bf))
    cos2 = np.ascontiguousarray(cos.reshape(S, DR).astype(np.float32))
    sin2 = np.ascontiguousarray(sin.reshape(S, DR).astype(np.float32))

    # de-interleave permutation for a 64-wide rope slice
    perm = np.concatenate([np.arange(0, DR, 2), np.arange(1, DR, 2)])

    wkva = np.array(wkv_a, copy=True)
    wkva[:, KVLR:] = wkva[:, KVLR:][:, perm]
    wkva = wkva.astype(bf)

    scale = np.float32(DQK) ** np.float32(-0.5)
    wqb = np.asarray(wq_b * q_ln[:, None] * scale)
    wqb = wqb.reshape(QLR, NH, DQK)
    wqb = np.concatenate([wqb[:, :, :DN], wqb[:, :, DN:][:, :, perm]],
                         axis=2).reshape(QLR, NH * DQK).astype(bf)

    wkvb = (wkv_b * kv_ln[:, None]).astype(bf)
    wob = wo.astype(bf)

    in_maps = []
    for c in range(NC):
        r = slice(c * SPC, (c + 1) * SPC)
        hcols = slice(c * HPC * (DN + DV), (c + 1) * HPC * (DN + DV))
        hrows = slice(c * HPC * DV, (c + 1) * HPC * DV)
        in_maps.append({
            "hsT": np.ascontiguousarray(hsT[:, r]),
            "cosr": np.ascontiguousarray(cos2[r]),
            "sinr": np.ascontiguousarray(sin2[r]),
            "wqa": wq_a.astype(bf),
            "wkva": wkva,
            "wqb": wqb,
            "wkvb": np.ascontiguousarray(wkvb[:, hcols]),
            "wo": np.ascontiguousarray(wob[hrows]),
        })
    return in_maps


def kernel(**inputs) -> np.ndarray:
    if "nc" not in _CACHED:
        _CACHED["nc"] = build()
    nc = _CACHED["nc"]
    in_maps = _prep(**inputs)
    res = run_bass_kernel_spmd(nc, in_maps, list(range(NC)))
    out = np.zeros((S, H), np.float32)
    for c in range(NC):
        out += res.results[c]["out"].astype(np.float32)
    return out.reshape(B, S, H)


if __name__ == "__main__":
    rng = np.random.RandomState(0)
    ins = {
        "hidden_states": rng.randn(B, S, H).astype(np.float32),
        "cos": rng.rand(B, S, DR).astype(np.float32),
        "sin": rng.rand(B, S, DR).astype(np.float32),
        "wq_a": (rng.randn(H, QLR) * 0.02).astype(np.float32),
        "q_ln": np.ones(QLR, np.float32),
        "wq_b": (rng.randn(QLR, NH * DQK) * 0.02).astype(np.float32),
        "wkv_a": (rng.randn(H, KVLR + DR) * 0.02).astype(np.float32),
        "kv_ln": np.ones(KVLR, np.float32),
        "wkv_b": (rng.randn(KVLR, NH * (DN + DV)) * 0.02).astype(np.float32),
        "wo": (rng.randn(NH * DV, H) * 0.02).astype(np.float32),
    }
    out = kernel(**ins)
    print("kernel out", out.shape, out.dtype, np.abs(out).mean())
